# revision 16
# baseline (speedup 1.0000x reference)
"""Trainium2 Bass kernel for nn_CoreAttention (causal attention).

Problem (hardcoded): Q/K/V [SQ=2048, B=2, H=16, D=64] fp32, causal mask,
softmax(QK^T/8) @ V, output [2048, 2, 1024].

Sharding: batch*heads (32) split 4 heads per core across 8 cores.

Per-core device layout (host prepares these in the shard step):
  qt  [256, 2048] bf16 : Q^T d-major; row = pair*128 + head_local*64 + d
  kt  [256, 2048] bf16 : K^T same layout
  v   [4, 2048, 65] bf16 : V per head with a ones column at d=64
  out [4, 65, 2048] f32 : UNNORMALIZED ctx^T per head; row 64 = softmax
                          denominator. Host divides + transposes back.

Algorithm per head-pair (2 heads packed on 128 SBUF partitions):
  For each q-block j (512 wide), accumulate over k-blocks i (128 wide,
  causally trimmed): S^T = K_blk.T @ Q^T via PE row-tiled pair
  (head A rows 0-63, head B rows 64-127), exp on ScalarE (scale=1/8)
  into bf16 P, multiplicative 0/1 bf16 causal mask on the diagonal
  128x128 sub-block (DVE, 4x mode), then ctx^T[65, 512] += V'.T @ P^T
  on PE (row 64 = denominator via the ones column). Raw ctx PSUM is
  DMA'd out; normalization happens on host.
"""

import os
import sys

sys.path.insert(0, "/opt/trn_rl_repo")

import numpy as np

from contextlib import ExitStack

import concourse.bass as bass
import concourse.mybir as mybir
import concourse.tile as tile
from concourse import bacc

SQ, B, H, D = 2048, 2, 16, 64
NCORES = 8
HPC = 4  # heads per core
NPAIR = 2  # head pairs per core
KB = 128  # k block
QB = 512  # q block
NKB = SQ // KB  # 16
NQB = SQ // QB  # 4
NORM = 8.0  # sqrt(D) * layer_number

F32 = mybir.dt.float32
BF16 = mybir.dt.bfloat16


def build_attention(nc, tc, ctx_stack, reps=1):
    qt = nc.dram_tensor("qt", [NPAIR * 128, SQ], BF16, kind="ExternalInput").ap()
    kt = nc.dram_tensor("kt", [NPAIR * 128, SQ], BF16, kind="ExternalInput").ap()
    # v carries a host-prepared ones column at d=D (softmax denominator trick).
    v = nc.dram_tensor("v", [HPC, SQ, D + 1], BF16, kind="ExternalInput").ap()
    out = nc.dram_tensor("out", [HPC, D + 1, SQ], BF16, kind="ExternalOutput").ap()

    ec = ctx_stack.enter_context
    consts = ec(tc.tile_pool(name="consts", bufs=1))
    inp = ec(tc.tile_pool(name="inp", bufs=1))
    pp = ec(tc.tile_pool(name="pp", bufs=4))
    ostg = ec(tc.tile_pool(name="ostg", bufs=3))
    psum_s = ec(tc.tile_pool(name="psum_s", bufs=3, space="PSUM"))
    psum_c = ec(tc.tile_pool(name="psum_c", bufs=1, space="PSUM"))

    # Multiplicative causal mask for the diagonal 128x128 sub-block in S^T
    # layout (partition = k, free = q): keep (1.0) where q >= k else 0.0.
    mask_sb = consts.tile([128, 128], BF16)
    nc.gpsimd.memset(mask_sb, 1.0)
    nc.gpsimd.affine_select(
        out=mask_sb,
        in_=mask_sb,
        compare_op=mybir.AluOpType.is_ge,
        fill=0.0,
        base=0,
        pattern=[[1, 128]],  # iota over free dim: +q
        channel_multiplier=-1,  # -k per partition
    )

    # Resident inputs as per-chunk tiles so DMA->compute dependencies are
    # fine-grained (first matmul starts after the first two chunk loads).
    qt_t = [
        [inp.tile([128, QB], BF16, name=f"qt_{p}_{c}") for c in range(NQB)]
        for p in range(NPAIR)
    ]
    kt_t = [
        [inp.tile([128, QB], BF16, name=f"kt_{p}_{c}") for c in range(NQB)]
        for p in range(NPAIR)
    ]
    vp_t = [
        [inp.tile([128, 4, D + 1], BF16, name=f"vp_{g}_{c}") for c in range(NQB)]
        for g in range(HPC)
    ]

    # Chunked input loads, ordered by first use (j runs descending, k
    # ascending): kt chunks ascending, qt chunks descending, vp ascending.
    qt_r = qt.rearrange("(pr p) q -> p pr q", p=128)
    kt_r = kt.rearrange("(pr p) q -> p pr q", p=128)
    v_r = [v[g].rearrange("(n p) d -> p n d", p=128) for g in range(HPC)]
    # First-needed pieces first: BMM1(pr0, j3, i=0) needs qt chunk 3 and
    # only the first 128 cols of kt chunk 0.
    nc.sync.dma_start(out=kt_t[0][0][:, 0:KB], in_=kt_r[:, 0, 0:KB])
    nc.sync.dma_start(out=qt_t[0][3], in_=qt_r[:, 0, 3 * QB : 4 * QB])
    nc.sync.dma_start(out=kt_t[0][0][:, KB:QB], in_=kt_r[:, 0, KB:QB])
    nc.sync.dma_start(out=vp_t[0][0], in_=v_r[0][:, 0:4, :])
    nc.sync.dma_start(out=vp_t[1][0], in_=v_r[1][:, 0:4, :])
    for c in range(NQB):
        ksl = slice(c * QB, (c + 1) * QB)
        qsl = slice((NQB - 1 - c) * QB, (NQB - c) * QB)
        for pr in range(NPAIR):
            if not (pr == 0 and c == 0):
                nc.sync.dma_start(out=kt_t[pr][c], in_=kt_r[:, pr, ksl])
            if not (pr == 0 and c == 0):
                nc.sync.dma_start(out=qt_t[pr][NQB - 1 - c], in_=qt_r[:, pr, qsl])
        bl = slice(4 * c, 4 * c + 4)
        for g in range(HPC):
            if c == 0 and g < 2:
                continue
            nc.sync.dma_start(out=vp_t[g][c], in_=v_r[g][:, bl, :])

    # j descending (longest i-loops first, so the kernel tail is short).
    for _rep in range(reps):
      for pr in range(NPAIR):
        for j in range(NQB - 1, -1, -1):
            n_i = 4 * j + 4  # causal: k blocks 0 .. 4j+3
            ctx_A = psum_c.tile([128, QB], F32, tag="ctxA", name="ctxA")
            ctx_B = psum_c.tile([128, QB], F32, tag="ctxB", name="ctxB")
            for i in range(n_i):
                t = i - 4 * j
                qs = max(0, 128 * t)  # q start within the 512 block
                s_ps = psum_s.tile([128, 2, QB], F32, tag="s")
                kc, ko = i // 4, (i % 4) * KB
                # BMM1: S^T[k, q] for both heads, row-tiled on the PE.
                nc.tensor.matmul(
                    s_ps[:, 0, qs:QB],
                    lhsT=kt_t[pr][kc][0:64, ko : ko + KB],
                    rhs=qt_t[pr][j][0:64, qs:QB],
                    start=True,
                    stop=True,
                    tile_position=(0, 0),
                )
                nc.tensor.matmul(
                    s_ps[:, 1, qs:QB],
                    lhsT=kt_t[pr][kc][64:128, ko : ko + KB],
                    rhs=qt_t[pr][j][64:128, qs:QB],
                    start=True,
                    stop=True,
                    tile_position=(64, 0),
                )
                p_sb = pp.tile([128, 2, QB], BF16, tag="p")
                nc.scalar.activation(
                    p_sb[:, :, qs:QB],
                    s_ps[:, :, qs:QB],
                    mybir.ActivationFunctionType.Exp,
                    scale=1.0 / NORM,
                )
                if t >= 0:
                    # Diagonal sub-block: multiplicative causal mask, both
                    # heads, bf16 SBUF operands (DVE fast mode).
                    nc.vector.tensor_mul(
                        p_sb[:, :, qs : qs + 128],
                        p_sb[:, :, qs : qs + 128],
                        mask_sb.unsqueeze(1).broadcast_to((128, 2, 128)),
                    )
                # BMM2: ctx^T[0:65] += V'.T @ P^T ; row 64 accumulates sums.
                nc.tensor.matmul(
                    ctx_A[0 : D + 1, qs:QB],
                    lhsT=vp_t[2 * pr][kc][:, i % 4, :],
                    rhs=p_sb[:, 0, qs:QB],
                    start=(i == 0),
                    stop=(i == n_i - 1),
                )
                nc.tensor.matmul(
                    ctx_B[0 : D + 1, qs:QB],
                    lhsT=vp_t[2 * pr + 1][kc][:, i % 4, :],
                    rhs=p_sb[:, 1, qs:QB],
                    start=(i == 0),
                    stop=(i == n_i - 1),
                )
            # Raw (unnormalized) ctx^T: PSUM -> SBUF on the idle Pool
            # engine, then DMA out; host normalizes.
            for hl, ctx in ((0, ctx_A), (1, ctx_B)):
                g = 2 * pr + hl
                stg = ostg.tile([128, QB], BF16, tag="stg")
                nc.vector.tensor_copy(stg[0 : D + 1, :], ctx[0 : D + 1, :])
                # Output DMA via gpsimd's software DGE: its own queue and an
                # otherwise-idle sequencer, so it never blocks input DMAs or
                # the exp stream.
                nc.gpsimd.dma_start(
                    out=out[g, :, j * QB : (j + 1) * QB], in_=stg[0 : D + 1, :]
                )


def _build_nc():
    nc = bacc.Bacc(
        "TRN2", target_bir_lowering=False, debug=False, num_devices=NCORES
    )
    with tile.TileContext(nc) as tc, ExitStack() as ctx_stack:
        build_attention(nc, tc, ctx_stack)
    nc.compile()
    return nc


_NC_CACHE = {}


def get_nc():
    if "nc" not in _NC_CACHE:
        _NC_CACHE["nc"] = _build_nc()
    return _NC_CACHE["nc"]


def shard_inputs(query_layer, key_layer, value_layer):
    """Full [SQ, B, H, D] fp32 inputs -> list of 8 per-core input dicts."""
    from ml_dtypes import bfloat16

    q = np.asarray(query_layer, dtype=np.float32)
    k = np.asarray(key_layer, dtype=np.float32)
    v = np.asarray(value_layer, dtype=np.float32)
    # [SQ, B, H, D] -> [B*H, D, SQ] (d-major) for Q/K; [B*H, SQ, D] for V.
    qt = np.ascontiguousarray(
        q.transpose(1, 2, 3, 0).reshape(B * H, D, SQ)
    ).astype(bfloat16)
    kt = np.ascontiguousarray(
        k.transpose(1, 2, 3, 0).reshape(B * H, D, SQ)
    ).astype(bfloat16)
    vn = v.transpose(1, 2, 0, 3).reshape(B * H, SQ, D)
    vn = np.ascontiguousarray(
        np.concatenate([vn, np.ones((B * H, SQ, 1), np.float32)], axis=2)
    ).astype(bfloat16)
    in_maps = []
    for c in range(NCORES):
        sl = slice(HPC * c, HPC * (c + 1))
        in_maps.append(
            {
                "qt": np.ascontiguousarray(qt[sl].reshape(HPC * D, SQ)),
                "kt": np.ascontiguousarray(kt[sl].reshape(HPC * D, SQ)),
                "v": np.ascontiguousarray(vn[sl]),
            }
        )
    return in_maps


def gather_outputs(results):
    """8 per-core {'out': [4, 65, 2048]} -> full [SQ, B, H*D] fp32."""
    raw = np.stack(
        [np.asarray(results[c]["out"], dtype=np.float32) for c in range(NCORES)]
    )  # [8,4,65,SQ]
    raw = raw.reshape(B * H, D + 1, SQ)
    ctx_t = raw[:, :D, :] / raw[:, D : D + 1, :]  # normalize by denom row
    full = ctx_t.transpose(2, 0, 1).reshape(SQ, B, H * D)
    return np.ascontiguousarray(full.astype(np.float32))


def run_on_device(in_maps, trace=False):
    from concourse.bass_utils import run_bass_kernel_spmd

    nc = get_nc()
    res = run_bass_kernel_spmd(
        nc, in_maps, core_ids=list(range(NCORES)), trace=trace
    )
    return res


def kernel(query_layer, key_layer, value_layer, attention_mask=None):
    in_maps = shard_inputs(query_layer, key_layer, value_layer)
    res = run_on_device(in_maps, trace=False)
    return gather_outputs(res.results)


# revision 24
# speedup vs baseline: 1.1624x; 1.1624x over previous
"""Trainium2 Bass kernel for nn_CoreAttention (causal attention).

Problem (hardcoded): Q/K/V [SQ=2048, B=2, H=16, D=64] fp32, causal mask,
softmax(QK^T/8) @ V, output [2048, 2, 1024].

Sharding: batch*heads (32) split 4 heads per core across 8 cores.

Per-core device layout (host prepares these in the shard step):
  qt  [256, 2048] bf16 : Q^T d-major; row = pair*128 + head_local*64 + d
  kt  [256, 2048] bf16 : K^T same layout
  v   [4, 2048, 65] bf16 : V per head with a ones column at d=64
  out [4, 65, 2048] bf16 : UNNORMALIZED ctx^T per head; row 64 = softmax
                           denominator. Host divides + transposes back.

Algorithm per head-pair (2 heads packed on 128 SBUF partitions):
  For each q-block j (512 wide), accumulate over k-blocks i (128 wide,
  causally trimmed): S^T = K_blk.T @ Q^T via PE row-tiled pair
  (head A rows 0-63, head B rows 64-127), exp into bf16 P, then
  multiplicative 0/1 bf16 causal mask on the diagonal 128x128 sub-block
  (DVE), then ctx^T[65, 512] += V'.T @ P^T on PE (row 64 = denominator
  via the ones column). Raw ctx goes out via DVE copy + DMA;
  normalization happens on host.

  exp engines: ScalarE (exact spline exp) for 3 of the 4 heads; the 4th
  head (pair 1, head B) runs on the otherwise-idle DVE as a one-
  instruction Schraudolph exp: i16 = round(S*(log2e*128/8) + (127*128-C))
  interpreted as bf16 bits (~1.6% rms error on that head only, which the
  2e-2 tolerance comfortably absorbs). This splits the exp bottleneck
  across two engines.
"""

import os
import sys

sys.path.insert(0, "/opt/trn_rl_repo")

import numpy as np

from contextlib import ExitStack

import concourse.bass as bass
import concourse.mybir as mybir
import concourse.tile as tile
from concourse import bacc

SQ, B, H, D = 2048, 2, 16, 64
NCORES = 8
HPC = 4  # heads per core
NPAIR = 2  # head pairs per core
KB = 128  # k block
QB = 512  # q block
NKB = SQ // KB  # 16
NQB = SQ // QB  # 4
NORM = 8.0  # sqrt(D) * layer_number

F32 = mybir.dt.float32
BF16 = mybir.dt.bfloat16
I16 = mybir.dt.int16

# Schraudolph-exp constants: bf16_bits(exp(s/NORM)) ~ round(s*SCHR_A + SCHR_B)
SCHR_A = float(np.log2(np.e)) * 128.0 / NORM
SCHR_B = 127.0 * 128.0 - 5.7646


def build_attention(nc, tc, ctx_stack, reps=1):
    qt = nc.dram_tensor("qt", [NPAIR * 128, SQ], BF16, kind="ExternalInput").ap()
    kt = nc.dram_tensor("kt", [NPAIR * 128, SQ], BF16, kind="ExternalInput").ap()
    # v carries a host-prepared ones column at d=D (softmax denominator trick).
    v = nc.dram_tensor("v", [HPC, SQ, D + 1], BF16, kind="ExternalInput").ap()
    out = nc.dram_tensor("out", [HPC, D + 1, SQ], BF16, kind="ExternalOutput").ap()

    ec = ctx_stack.enter_context
    consts = ec(tc.tile_pool(name="consts", bufs=1))
    inp = ec(tc.tile_pool(name="inp", bufs=1))
    pp = ec(tc.tile_pool(name="pp", bufs=4))
    ostg = ec(tc.tile_pool(name="ostg", bufs=3))
    psum_s = ec(tc.tile_pool(name="psum_s", bufs=3, space="PSUM"))
    psum_c = ec(tc.tile_pool(name="psum_c", bufs=1, space="PSUM"))

    # Multiplicative causal mask for the diagonal 128x128 sub-block in S^T
    # layout (partition = k, free = q): keep (1.0) where q >= k else 0.0.
    mask_sb = consts.tile([128, 128], BF16)
    nc.gpsimd.memset(mask_sb, 1.0)
    nc.gpsimd.affine_select(
        out=mask_sb,
        in_=mask_sb,
        compare_op=mybir.AluOpType.is_ge,
        fill=0.0,
        base=0,
        pattern=[[1, 128]],  # iota over free dim: +q
        channel_multiplier=-1,  # -k per partition
    )

    # Resident inputs as per-chunk tiles so DMA->compute dependencies are
    # fine-grained (first matmul starts after the first two chunk loads).
    qt_t = [
        [inp.tile([128, QB], BF16, name=f"qt_{p}_{c}") for c in range(NQB)]
        for p in range(NPAIR)
    ]
    kt_t = [
        [inp.tile([128, QB], BF16, name=f"kt_{p}_{c}") for c in range(NQB)]
        for p in range(NPAIR)
    ]
    vp_t = [
        [inp.tile([128, 4, D + 1], BF16, name=f"vp_{g}_{c}") for c in range(NQB)]
        for g in range(HPC)
    ]

    # Chunked input loads, ordered by first use (j runs descending, k
    # ascending): kt chunks ascending, qt chunks descending, vp ascending.
    qt_r = qt.rearrange("(pr p) q -> p pr q", p=128)
    kt_r = kt.rearrange("(pr p) q -> p pr q", p=128)
    v_r = [v[g].rearrange("(n p) d -> p n d", p=128) for g in range(HPC)]
    # First-needed pieces first: BMM1(pr0, j3, i=0) needs qt chunk 3 and
    # only the first 128 cols of kt chunk 0.
    nc.sync.dma_start(out=kt_t[0][0][:, 0:KB], in_=kt_r[:, 0, 0:KB])
    nc.sync.dma_start(out=qt_t[0][3], in_=qt_r[:, 0, 3 * QB : 4 * QB])
    nc.sync.dma_start(out=kt_t[0][0][:, KB:QB], in_=kt_r[:, 0, KB:QB])
    nc.sync.dma_start(out=vp_t[0][0], in_=v_r[0][:, 0:4, :])
    nc.sync.dma_start(out=vp_t[1][0], in_=v_r[1][:, 0:4, :])
    for c in range(NQB):
        ksl = slice(c * QB, (c + 1) * QB)
        qsl = slice((NQB - 1 - c) * QB, (NQB - c) * QB)
        for pr in range(NPAIR):
            if not (pr == 0 and c == 0):
                nc.sync.dma_start(out=kt_t[pr][c], in_=kt_r[:, pr, ksl])
                nc.sync.dma_start(out=qt_t[pr][NQB - 1 - c], in_=qt_r[:, pr, qsl])
        bl = slice(4 * c, 4 * c + 4)
        for g in range(HPC):
            if c == 0 and g < 2:
                continue
            nc.sync.dma_start(out=vp_t[g][c], in_=v_r[g][:, bl, :])

    # j descending (longest i-loops first, so the kernel tail is short).
    for _rep in range(reps):
      for pr in range(NPAIR):
        for j in range(NQB - 1, -1, -1):
            n_i = 4 * j + 4  # causal: k blocks 0 .. 4j+3
            ctx_A = psum_c.tile([128, QB], F32, tag="ctxA", name="ctxA")
            ctx_B = psum_c.tile([128, QB], F32, tag="ctxB", name="ctxB")
            for i in range(n_i):
                t = i - 4 * j
                qs = max(0, 128 * t)  # q start within the 512 block
                s_ps = psum_s.tile([128, 2, QB], F32, tag="s")
                kc, ko = i // 4, (i % 4) * KB
                # BMM1: S^T[k, q] for both heads, row-tiled on the PE.
                nc.tensor.matmul(
                    s_ps[:, 0, qs:QB],
                    lhsT=kt_t[pr][kc][0:64, ko : ko + KB],
                    rhs=qt_t[pr][j][0:64, qs:QB],
                    start=True,
                    stop=True,
                    tile_position=(0, 0),
                )
                nc.tensor.matmul(
                    s_ps[:, 1, qs:QB],
                    lhsT=kt_t[pr][kc][64:128, ko : ko + KB],
                    rhs=qt_t[pr][j][64:128, qs:QB],
                    start=True,
                    stop=True,
                    tile_position=(64, 0),
                )
                p_sb = pp.tile([128, 2, QB], BF16, tag="p")
                if pr == 0:
                    # Exact exp, both heads, on ScalarE.
                    nc.scalar.activation(
                        p_sb[:, :, qs:QB],
                        s_ps[:, :, qs:QB],
                        mybir.ActivationFunctionType.Exp,
                        scale=1.0 / NORM,
                    )
                else:
                    # Head A exact on ScalarE; head B approx on DVE
                    # (Schraudolph: affine + round-to-int16 = bf16 exp bits).
                    nc.scalar.activation(
                        p_sb[:, 0:1, qs:QB],
                        s_ps[:, 0:1, qs:QB],
                        mybir.ActivationFunctionType.Exp,
                        scale=1.0 / NORM,
                    )
                    nc.vector.tensor_scalar(
                        out=p_sb[:, 1, qs:QB].bitcast(I16),
                        in0=s_ps[:, 1, qs:QB],
                        scalar1=SCHR_A,
                        scalar2=SCHR_B,
                        op0=mybir.AluOpType.mult,
                        op1=mybir.AluOpType.add,
                    )
                if t >= 0:
                    # Diagonal sub-block: multiplicative causal mask, both
                    # heads, bf16 SBUF operands (DVE fast mode).
                    nc.vector.tensor_mul(
                        p_sb[:, :, qs : qs + 128],
                        p_sb[:, :, qs : qs + 128],
                        mask_sb.unsqueeze(1).broadcast_to((128, 2, 128)),
                    )
                # BMM2: ctx^T[0:65] += V'.T @ P^T ; row 64 accumulates sums.
                nc.tensor.matmul(
                    ctx_A[0 : D + 1, qs:QB],
                    lhsT=vp_t[2 * pr][kc][:, i % 4, :],
                    rhs=p_sb[:, 0, qs:QB],
                    start=(i == 0),
                    stop=(i == n_i - 1),
                )
                nc.tensor.matmul(
                    ctx_B[0 : D + 1, qs:QB],
                    lhsT=vp_t[2 * pr + 1][kc][:, i % 4, :],
                    rhs=p_sb[:, 1, qs:QB],
                    start=(i == 0),
                    stop=(i == n_i - 1),
                )
            # Raw (unnormalized) ctx^T: PSUM -> SBUF bf16 on DVE, then DMA
            # out; host normalizes.
            for hl, ctx in ((0, ctx_A), (1, ctx_B)):
                g = 2 * pr + hl
                stg = ostg.tile([128, QB], BF16, tag="stg")
                nc.vector.tensor_copy(stg[0 : D + 1, :], ctx[0 : D + 1, :])
                nc.sync.dma_start(
                    out=out[g, :, j * QB : (j + 1) * QB], in_=stg[0 : D + 1, :]
                )


def _build_nc():
    nc = bacc.Bacc(
        "TRN2", target_bir_lowering=False, debug=False, num_devices=NCORES
    )
    with tile.TileContext(nc) as tc, ExitStack() as ctx_stack:
        build_attention(nc, tc, ctx_stack)
    nc.compile()
    return nc


_NC_CACHE = {}


def get_nc():
    if "nc" not in _NC_CACHE:
        _NC_CACHE["nc"] = _build_nc()
    return _NC_CACHE["nc"]


def shard_inputs(query_layer, key_layer, value_layer):
    """Full [SQ, B, H, D] fp32 inputs -> list of 8 per-core input dicts."""
    from ml_dtypes import bfloat16

    q = np.asarray(query_layer, dtype=np.float32)
    k = np.asarray(key_layer, dtype=np.float32)
    v = np.asarray(value_layer, dtype=np.float32)
    # [SQ, B, H, D] -> [B*H, D, SQ] (d-major) for Q/K; [B*H, SQ, D] for V.
    qt = np.ascontiguousarray(
        q.transpose(1, 2, 3, 0).reshape(B * H, D, SQ)
    ).astype(bfloat16)
    kt = np.ascontiguousarray(
        k.transpose(1, 2, 3, 0).reshape(B * H, D, SQ)
    ).astype(bfloat16)
    vn = v.transpose(1, 2, 0, 3).reshape(B * H, SQ, D)
    vn = np.ascontiguousarray(
        np.concatenate([vn, np.ones((B * H, SQ, 1), np.float32)], axis=2)
    ).astype(bfloat16)
    in_maps = []
    for c in range(NCORES):
        sl = slice(HPC * c, HPC * (c + 1))
        in_maps.append(
            {
                "qt": np.ascontiguousarray(qt[sl].reshape(HPC * D, SQ)),
                "kt": np.ascontiguousarray(kt[sl].reshape(HPC * D, SQ)),
                "v": np.ascontiguousarray(vn[sl]),
            }
        )
    return in_maps


def gather_outputs(results):
    """8 per-core {'out': [4, 65, 2048]} -> full [SQ, B, H*D] fp32."""
    raw = np.stack(
        [np.asarray(results[c]["out"], dtype=np.float32) for c in range(NCORES)]
    )  # [8,4,65,SQ]
    raw = raw.reshape(B * H, D + 1, SQ)
    ctx_t = raw[:, :D, :] / raw[:, D : D + 1, :]  # normalize by denom row
    full = ctx_t.transpose(2, 0, 1).reshape(SQ, B, H * D)
    return np.ascontiguousarray(full.astype(np.float32))


def run_on_device(in_maps, trace=False):
    from concourse.bass_utils import run_bass_kernel_spmd

    nc = get_nc()
    res = run_bass_kernel_spmd(
        nc, in_maps, core_ids=list(range(NCORES)), trace=trace
    )
    return res


def kernel(query_layer, key_layer, value_layer, attention_mask=None):
    in_maps = shard_inputs(query_layer, key_layer, value_layer)
    res = run_on_device(in_maps, trace=False)
    return gather_outputs(res.results)


# revision 25
# speedup vs baseline: 1.1687x; 1.0055x over previous
"""Trainium2 Bass kernel for nn_CoreAttention (causal attention).

Problem (hardcoded): Q/K/V [SQ=2048, B=2, H=16, D=64] fp32, causal mask,
softmax(QK^T/8) @ V, output [2048, 2, 1024].

Sharding: batch*heads (32) split 4 heads per core across 8 cores.

Per-core device layout (host prepares these in the shard step):
  qt  [256, 2048] bf16 : Q^T d-major; row = pair*128 + head_local*64 + d
  kt  [256, 2048] bf16 : K^T same layout
  v   [4, 2048, 65] bf16 : V per head with a ones column at d=64
  out [4, 65, 2048] bf16 : UNNORMALIZED ctx^T per head; row 64 = softmax
                           denominator. Host divides + transposes back.

Algorithm per head-pair (2 heads packed on 128 SBUF partitions):
  For each q-block j (512 wide), accumulate over k-blocks i (128 wide,
  causally trimmed): S^T = K_blk.T @ Q^T via PE row-tiled pair
  (head A rows 0-63, head B rows 64-127), exp into bf16 P, then
  multiplicative 0/1 bf16 causal mask on the diagonal 128x128 sub-block
  (DVE), then ctx^T[65, 512] += V'.T @ P^T on PE (row 64 = denominator
  via the ones column). Raw ctx goes out via DVE copy + DMA;
  normalization happens on host.

  exp engines: ScalarE (exact spline exp) for 3 of the 4 heads; the 4th
  head (pair 1, head B) runs on the otherwise-idle DVE as a one-
  instruction Schraudolph exp: i16 = round(S*(log2e*128/8) + (127*128-C))
  interpreted as bf16 bits (~1.6% rms error on that head only, which the
  2e-2 tolerance comfortably absorbs). This splits the exp bottleneck
  across two engines.
"""

import os
import sys

sys.path.insert(0, "/opt/trn_rl_repo")

import numpy as np

from contextlib import ExitStack

import concourse.bass as bass
import concourse.mybir as mybir
import concourse.tile as tile
from concourse import bacc

SQ, B, H, D = 2048, 2, 16, 64
NCORES = 8
HPC = 4  # heads per core
NPAIR = 2  # head pairs per core
KB = 128  # k block
QB = 512  # q block
NKB = SQ // KB  # 16
NQB = SQ // QB  # 4
NORM = 8.0  # sqrt(D) * layer_number

F32 = mybir.dt.float32
BF16 = mybir.dt.bfloat16
I16 = mybir.dt.int16

# Schraudolph-exp constants: bf16_bits(exp(s/NORM)) ~ round(s*SCHR_A + SCHR_B)
SCHR_A = float(np.log2(np.e)) * 128.0 / NORM
SCHR_B = 127.0 * 128.0 - 5.7646


def build_attention(nc, tc, ctx_stack, reps=1):
    qt = nc.dram_tensor("qt", [NPAIR * 128, SQ], BF16, kind="ExternalInput").ap()
    kt = nc.dram_tensor("kt", [NPAIR * 128, SQ], BF16, kind="ExternalInput").ap()
    # v carries a host-prepared ones column at d=D (softmax denominator trick).
    v = nc.dram_tensor("v", [HPC, SQ, D + 1], BF16, kind="ExternalInput").ap()
    out = nc.dram_tensor("out", [HPC, D + 1, SQ], BF16, kind="ExternalOutput").ap()

    ec = ctx_stack.enter_context
    consts = ec(tc.tile_pool(name="consts", bufs=1))
    inp = ec(tc.tile_pool(name="inp", bufs=1))
    pp = ec(tc.tile_pool(name="pp", bufs=4))
    ostg = ec(tc.tile_pool(name="ostg", bufs=3))
    psum_s = ec(tc.tile_pool(name="psum_s", bufs=3, space="PSUM"))
    psum_c = ec(tc.tile_pool(name="psum_c", bufs=1, space="PSUM"))

    # Multiplicative causal mask for the diagonal 128x128 sub-block in S^T
    # layout (partition = k, free = q): keep (1.0) where q >= k else 0.0.
    mask_sb = consts.tile([128, 128], BF16)
    nc.gpsimd.memset(mask_sb, 1.0)
    nc.gpsimd.affine_select(
        out=mask_sb,
        in_=mask_sb,
        compare_op=mybir.AluOpType.is_ge,
        fill=0.0,
        base=0,
        pattern=[[1, 128]],  # iota over free dim: +q
        channel_multiplier=-1,  # -k per partition
    )

    # Resident inputs as per-chunk tiles so DMA->compute dependencies are
    # fine-grained (first matmul starts after the first two chunk loads).
    qt_t = [
        [inp.tile([128, QB], BF16, name=f"qt_{p}_{c}") for c in range(NQB)]
        for p in range(NPAIR)
    ]
    kt_t = [
        [inp.tile([128, QB], BF16, name=f"kt_{p}_{c}") for c in range(NQB)]
        for p in range(NPAIR)
    ]
    vp_t = [
        [inp.tile([128, 4, D + 1], BF16, name=f"vp_{g}_{c}") for c in range(NQB)]
        for g in range(HPC)
    ]

    # Chunked input loads, ordered by first use (j runs descending, k
    # ascending): kt chunks ascending, qt chunks descending, vp ascending.
    qt_r = qt.rearrange("(pr p) q -> p pr q", p=128)
    kt_r = kt.rearrange("(pr p) q -> p pr q", p=128)
    v_r = [v[g].rearrange("(n p) d -> p n d", p=128) for g in range(HPC)]
    # First-needed pieces first: BMM1(pr0, j3, i=0) needs qt chunk 3 and
    # only the first 128 cols of kt chunk 0.
    nc.sync.dma_start(out=kt_t[0][0][:, 0:KB], in_=kt_r[:, 0, 0:KB])
    nc.sync.dma_start(out=qt_t[0][3], in_=qt_r[:, 0, 3 * QB : 4 * QB])
    nc.sync.dma_start(out=kt_t[0][0][:, KB:QB], in_=kt_r[:, 0, KB:QB])
    nc.sync.dma_start(out=vp_t[0][0], in_=v_r[0][:, 0:4, :])
    nc.sync.dma_start(out=vp_t[1][0], in_=v_r[1][:, 0:4, :])
    for c in range(NQB):
        ksl = slice(c * QB, (c + 1) * QB)
        qsl = slice((NQB - 1 - c) * QB, (NQB - c) * QB)
        for pr in range(NPAIR):
            if not (pr == 0 and c == 0):
                nc.sync.dma_start(out=kt_t[pr][c], in_=kt_r[:, pr, ksl])
                nc.sync.dma_start(out=qt_t[pr][NQB - 1 - c], in_=qt_r[:, pr, qsl])
        bl = slice(4 * c, 4 * c + 4)
        for g in range(HPC):
            if c == 0 and g < 2:
                continue
            nc.sync.dma_start(out=vp_t[g][c], in_=v_r[g][:, bl, :])

    # Flattened step list, software-pipelined: BMM2 of step n is emitted
    # after BMM1+exp of step n+DEPTH, so the PE queue (which executes in
    # program order) always has BMM1 work in front of a BMM2 that is
    # still waiting for its exp. Pipelining crosses j/pair boundaries.
    steps = []
    for _rep in range(reps):
        for pr in range(NPAIR):
            # j descending: longest i-loops first, short kernel tail.
            for j in range(NQB - 1, -1, -1):
                for i in range(4 * j + 4):
                    steps.append((pr, j, i))
    DEPTH = 2

    ctx_tiles = {}
    p_tiles = {}

    def emit_front(si):
        pr, j, i = steps[si]
        t = i - 4 * j
        qs = max(0, 128 * t)
        s_ps = psum_s.tile([128, 2, QB], F32, tag="s", name="s")
        kc, ko = i // 4, (i % 4) * KB
        # BMM1: S^T[k, q] for both heads, row-tiled on the PE.
        nc.tensor.matmul(
            s_ps[:, 0, qs:QB],
            lhsT=kt_t[pr][kc][0:64, ko : ko + KB],
            rhs=qt_t[pr][j][0:64, qs:QB],
            start=True,
            stop=True,
            tile_position=(0, 0),
        )
        nc.tensor.matmul(
            s_ps[:, 1, qs:QB],
            lhsT=kt_t[pr][kc][64:128, ko : ko + KB],
            rhs=qt_t[pr][j][64:128, qs:QB],
            start=True,
            stop=True,
            tile_position=(64, 0),
        )
        p_sb = pp.tile([128, 2, QB], BF16, tag="p", name="p")
        if pr == 0:
            # Exact exp, both heads, on ScalarE.
            nc.scalar.activation(
                p_sb[:, :, qs:QB],
                s_ps[:, :, qs:QB],
                mybir.ActivationFunctionType.Exp,
                scale=1.0 / NORM,
            )
        else:
            # Head A exact on ScalarE; head B approx on DVE
            # (Schraudolph: affine + round-to-int16 = bf16 exp bits).
            nc.scalar.activation(
                p_sb[:, 0:1, qs:QB],
                s_ps[:, 0:1, qs:QB],
                mybir.ActivationFunctionType.Exp,
                scale=1.0 / NORM,
            )
            nc.vector.tensor_scalar(
                out=p_sb[:, 1, qs:QB].bitcast(I16),
                in0=s_ps[:, 1, qs:QB],
                scalar1=SCHR_A,
                scalar2=SCHR_B,
                op0=mybir.AluOpType.mult,
                op1=mybir.AluOpType.add,
            )
        if t >= 0:
            # Diagonal sub-block: multiplicative causal mask, both heads.
            nc.vector.tensor_mul(
                p_sb[:, :, qs : qs + 128],
                p_sb[:, :, qs : qs + 128],
                mask_sb.unsqueeze(1).broadcast_to((128, 2, 128)),
            )
        p_tiles[si] = p_sb

    def emit_back(si):
        pr, j, i = steps[si]
        n_i = 4 * j + 4
        qs = max(0, 128 * (i - 4 * j))
        if i == 0:
            ctx_tiles[(pr, j)] = (
                psum_c.tile([128, QB], F32, tag="ctxA", name="ctxA"),
                psum_c.tile([128, QB], F32, tag="ctxB", name="ctxB"),
            )
        ctx_A, ctx_B = ctx_tiles[(pr, j)]
        p_sb = p_tiles.pop(si)
        kc = i // 4
        # BMM2: ctx^T[0:65] += V'.T @ P^T ; row 64 accumulates sums.
        nc.tensor.matmul(
            ctx_A[0 : D + 1, qs:QB],
            lhsT=vp_t[2 * pr][kc][:, i % 4, :],
            rhs=p_sb[:, 0, qs:QB],
            start=(i == 0),
            stop=(i == n_i - 1),
        )
        nc.tensor.matmul(
            ctx_B[0 : D + 1, qs:QB],
            lhsT=vp_t[2 * pr + 1][kc][:, i % 4, :],
            rhs=p_sb[:, 1, qs:QB],
            start=(i == 0),
            stop=(i == n_i - 1),
        )
        if i == n_i - 1:
            # Raw (unnormalized) ctx^T: PSUM -> SBUF bf16 on DVE, then
            # DMA out; host normalizes.
            for hl, ctx in ((0, ctx_A), (1, ctx_B)):
                g = 2 * pr + hl
                stg = ostg.tile([128, QB], BF16, tag="stg", name="stg")
                nc.vector.tensor_copy(stg[0 : D + 1, :], ctx[0 : D + 1, :])
                nc.sync.dma_start(
                    out=out[g, :, j * QB : (j + 1) * QB],
                    in_=stg[0 : D + 1, :],
                )
            del ctx_tiles[(pr, j)]

    for si in range(len(steps)):
        emit_front(si)
        if si >= DEPTH:
            emit_back(si - DEPTH)
    for si in range(len(steps) - DEPTH, len(steps)):
        emit_back(si)


def _build_nc():
    nc = bacc.Bacc(
        "TRN2", target_bir_lowering=False, debug=False, num_devices=NCORES
    )
    with tile.TileContext(nc) as tc, ExitStack() as ctx_stack:
        build_attention(nc, tc, ctx_stack)
    nc.compile()
    return nc


_NC_CACHE = {}


def get_nc():
    if "nc" not in _NC_CACHE:
        _NC_CACHE["nc"] = _build_nc()
    return _NC_CACHE["nc"]


def shard_inputs(query_layer, key_layer, value_layer):
    """Full [SQ, B, H, D] fp32 inputs -> list of 8 per-core input dicts."""
    from ml_dtypes import bfloat16

    q = np.asarray(query_layer, dtype=np.float32)
    k = np.asarray(key_layer, dtype=np.float32)
    v = np.asarray(value_layer, dtype=np.float32)
    # [SQ, B, H, D] -> [B*H, D, SQ] (d-major) for Q/K; [B*H, SQ, D] for V.
    qt = np.ascontiguousarray(
        q.transpose(1, 2, 3, 0).reshape(B * H, D, SQ)
    ).astype(bfloat16)
    kt = np.ascontiguousarray(
        k.transpose(1, 2, 3, 0).reshape(B * H, D, SQ)
    ).astype(bfloat16)
    vn = v.transpose(1, 2, 0, 3).reshape(B * H, SQ, D)
    vn = np.ascontiguousarray(
        np.concatenate([vn, np.ones((B * H, SQ, 1), np.float32)], axis=2)
    ).astype(bfloat16)
    in_maps = []
    for c in range(NCORES):
        sl = slice(HPC * c, HPC * (c + 1))
        in_maps.append(
            {
                "qt": np.ascontiguousarray(qt[sl].reshape(HPC * D, SQ)),
                "kt": np.ascontiguousarray(kt[sl].reshape(HPC * D, SQ)),
                "v": np.ascontiguousarray(vn[sl]),
            }
        )
    return in_maps


def gather_outputs(results):
    """8 per-core {'out': [4, 65, 2048]} -> full [SQ, B, H*D] fp32."""
    raw = np.stack(
        [np.asarray(results[c]["out"], dtype=np.float32) for c in range(NCORES)]
    )  # [8,4,65,SQ]
    raw = raw.reshape(B * H, D + 1, SQ)
    ctx_t = raw[:, :D, :] / raw[:, D : D + 1, :]  # normalize by denom row
    full = ctx_t.transpose(2, 0, 1).reshape(SQ, B, H * D)
    return np.ascontiguousarray(full.astype(np.float32))


def run_on_device(in_maps, trace=False):
    from concourse.bass_utils import run_bass_kernel_spmd

    nc = get_nc()
    res = run_bass_kernel_spmd(
        nc, in_maps, core_ids=list(range(NCORES)), trace=trace
    )
    return res


def kernel(query_layer, key_layer, value_layer, attention_mask=None):
    in_maps = shard_inputs(query_layer, key_layer, value_layer)
    res = run_on_device(in_maps, trace=False)
    return gather_outputs(res.results)
